# revision 4
# baseline (speedup 1.0000x reference)
"""MatchingNetwork forward on 8 Trainium2 NeuronCores.

The reference network's output reduces exactly to one_hot(labels, V) in f32:
the final einsum('btn,btv->btv', att, one_hot) sums att over n, and att is a
softmax over n, so the output is one_hot scaled by sum(softmax) == 1 (to float
rounding, ~1e-7).  Everything upstream (embedding gathers, BiLSTM GLayer,
attentional FLayer) cancels out of the result for every input.

So the kernel is a distributed one-hot materialization: B*T = 2048 rows of
V = 32000 each, data-parallel over rows across 8 cores (256 rows/core).
The values are exactly 0/1, so the device materializes the tensor as one
byte per element (8.19 MB/core instead of 32.77 MB in f32) and the host
upcasts to f32 on gather; HBM write traffic, the sole bottleneck
(~360-400 GB/s per core), drops 4x.

Byte pairs are packed into int16 elements so the DVE compare runs in the
packed 2-byte modes: at most one of two adjacent columns holds a 1, so
  pair[j] = (j == label>>1) * (label even ? 1 : 256)
emits little-endian bytes (lo, hi) = (onehot[2j], onehot[2j+1]) in a single
tensor_scalar(is_equal, mult) per tile, with per-partition [128,1] f32
scalar operands (label>>1 minus the chunk base, and the even/odd scale),
staged packed in ONE small input DMA.  The iota is generated on gpsimd (in
pieces, narrow leading chunks) so compares start as early as possible.
Each chunk's two row-batches go out in a single DMA via a transposed DRAM
access pattern; tapered tail chunks shorten the drain.
"""

import os
import sys

for _p in ("/opt/trn_rl_repo", "/root/.axon_site/_ro/trn_rl_repo"):
    if os.path.isdir(_p) and _p not in sys.path:
        sys.path.append(_p)

import numpy as np

B, T, V = 32, 64, 32000
N_CORES = 8
ROWS = B * T                 # 2048 one-hot rows total
RPC = ROWS // N_CORES        # 256 rows per core
NB = RPC // 128              # 2 batches of 128 partitions
VH = V // 2                  # 16000 int16 pairs per row
# Column-chunk widths in pair units: narrow leading chunks start compares
# after a short gpsimd iota piece; tapered tail shortens the final drain.
CHUNKS_H = [500, 500, 1000, 2000, 2000, 2000, 2000, 2000, 2000, 1500, 500]
assert sum(CHUNKS_H) == VH
NCHUNK = len(CHUNKS_H)
MAXH = max(CHUNKS_H)
IOTA_PIECES = [(0, 500), (500, 1000), (1000, 2000)]
NLAB = NB * NCHUNK + NB      # packed labm + scl columns

_cache = {}


def _build_nc():
    import concourse.bacc as bacc
    import concourse.mybir as mybir
    from concourse.tile import TileContext

    nc = bacc.Bacc()
    lab_d = nc.dram_tensor("lab", [128, NLAB], mybir.dt.float32,
                           kind="ExternalInput")
    out_d = nc.dram_tensor("out", [NB, 128, VH], mybir.dt.int16,
                           kind="ExternalOutput")

    with TileContext(nc) as tc:
        with tc.tile_pool(name="const", bufs=1) as cpool, \
             tc.tile_pool(name="work", bufs=6) as wpool:
            lab = cpool.tile([128, NLAB], mybir.dt.float32, tag="lab")
            nc.sync.dma_start(out=lab[:, :], in_=lab_d[:, :])
            iota = cpool.tile([128, MAXH], mybir.dt.int16, tag="iota")
            for (s, e) in IOTA_PIECES:
                nc.gpsimd.iota(iota[:, s:e], [[1, e - s]], base=s,
                               channel_multiplier=0)
            dma_engines = [nc.sync, nc.scalar]
            col = 0
            for ci, w in enumerate(CHUNKS_H):
                o = wpool.tile([128, 2 * MAXH], mybir.dt.int16, tag="o")
                for b in range(NB):
                    # o = (iota == (label>>1) - chunk_base) * (1 or 256)
                    nc.vector.tensor_scalar(
                        out=o[:, b * w:(b + 1) * w], in0=iota[:, :w],
                        scalar1=lab[:, b * NCHUNK + ci:b * NCHUNK + ci + 1],
                        scalar2=lab[:, NB * NCHUNK + b:NB * NCHUNK + b + 1],
                        op0=mybir.AluOpType.is_equal,
                        op1=mybir.AluOpType.mult)
                # one DMA for both row-batches: SBUF [128, (b w)] ->
                # DRAM [b, 128, w] with the partition dim outermost
                src = o[:, :2 * w].rearrange("p (b w) -> p b w", b=NB)
                dst = out_d[:, :, col:col + w].transpose([1, 0, 2])
                dma_engines[ci % 2].dma_start(out=dst, in_=src)
                col += w
    nc.finalize()
    return nc


def kernel(**inputs):
    from concourse.bass_utils import run_bass_kernel_spmd

    if "nc" not in _cache:
        _cache["nc"] = _build_nc()
    nc = _cache["nc"]

    lab = np.asarray(inputs["labels"]).reshape(-1).astype(np.int64)
    bases = np.cumsum([0] + CHUNKS_H[:-1]).astype(np.int64)  # [NCHUNK]
    in_maps = []
    for i in range(N_CORES):
        shard = lab[i * RPC:(i + 1) * RPC].reshape(NB, 128)   # [b, p]
        half = shard >> 1                                     # label // 2
        # packed[p, b*NCHUNK + c] = (label >> 1) - chunk_base[c]
        # packed[p, NB*NCHUNK + b] = 1 if label even else 256
        labm = (half[:, :, None] - bases[None, None, :])      # [b, p, c]
        labm = labm.transpose(1, 0, 2).reshape(128, NB * NCHUNK)
        scl = np.where(shard & 1, 256, 1).T                   # [p, b]
        packed = np.concatenate([labm, scl], axis=1).astype(np.float32)
        in_maps.append({"lab": np.ascontiguousarray(packed)})

    trace = bool(int(os.environ.get("BASS_KERNEL_TRACE", "0")))
    res = run_bass_kernel_spmd(nc, in_maps, list(range(N_CORES)), trace=trace)
    _cache["last_res"] = res

    outs = [np.asarray(res.results[i]["out"]).reshape(NB * 128, VH)
            for i in range(N_CORES)]
    packed = np.concatenate(outs, axis=0)                     # [ROWS, VH] i16
    return packed.view(np.uint8).astype(np.float32).reshape(B, T, V)
